# revision 1
# baseline (speedup 1.0000x reference)
"""Concatenation (additive/Bahdanau-style) attention Trainium2 kernel.

Math (per batch b):
    f = x @ W1[:H]          # [S, A]
    g = x @ W1[H:]          # [S, A]
    scores[i, j] = w2 . tanh(f[i] + g[j] + b1) + b2
    e = exp(scores) * (j < i)
    out[i] = sum_j e[i, j] x[j] / (sum_j e[i, j] + 1e-10)

Sharding: data-parallel over batch, one batch element per NeuronCore (B=8).
Everything stays on-chip; the [S, S, A] pairwise tensor never exists in HBM.

Per-core layout strategy:
  - j-block = 8 consecutive j values; partitions of the tanh tile hold
    (j8, a) pairs: p = 8*16 grid = j8*16 + a  (8 j's x 16 hidden units).
  - FB[p, i] = f[i, p%16] replicated 8x on partitions (one PE matmul with a
    host-replicated W1a).
  - G[p, jb] = g[8*jb + j8, a] + b1[a] with p = j8*16+a  (8 strided matmuls).
  - U[p, i] = FB[p, i] + G[p, jb]  (DVE tensor_scalar add, per-partition
    scalar) for the exact triangular range i >= 8*jb, then one big ACT tanh
    per group of 8 j-blocks (amortizes ACT fixed overhead).
  - scores via PE matmul with block-diag W2BD[p, m] = (p//16==m) * w2[p%16]
    contracting all 128 partitions -> [8 j, i] rows in PSUM; 16 j-blocks
    fill a [128, Lg] PSUM supertile (j = 128*g + p).
  - one ACT exp (bias=b2) per supertile PSUM -> SBUF e-tile; strictly-upper
    [128,128] mask on the diagonal chunk enforces j < i.
  - out: for each 128-row i-block, accumulate matmuls over supertiles g<=ib:
    lhsT = e_g[:, i-cols] (K=j), rhs = x_aug (x with a ones column) so the
    softmax denominator falls out of the same matmuls; then reciprocal+scale.
"""

import numpy as np

import concourse.bass as bass
import concourse.tile as tile
from concourse import bacc, mybir
from concourse.bass_utils import run_bass_kernel_spmd

B, S, H, A = 8, 1024, 128, 16
NCORES = 8
XAUG_W = H + 4  # x plus a ones column, padded to 132 floats (528 B)
NBLK = S // 8  # 128 j-blocks of 8

FT = mybir.ActivationFunctionType
F32 = mybir.dt.float32
BF16 = mybir.dt.float16  # fp16: same 1 col/cycle as bf16, 8x the mantissa

# Score-matmul dtype knob: bf16 streams 1 col/cycle (vs 4 for float32).
# (float32r also streams 1 col/cycle but requires dst partition 0 and is
# no more precise than bf16 on TRN2.)
SCORE_BF16 = True


def _build_nc():
    nc = bacc.Bacc(None)

    xaug_d = nc.declare_dram_parameter("x_aug", [S, XAUG_W], BF16, isOutput=False)
    xT_d = nc.declare_dram_parameter("xT", [H, S], BF16, isOutput=False)
    w1ra_d = nc.declare_dram_parameter("W1repA", [H, 128], BF16, isOutput=False)
    w1b_d = nc.declare_dram_parameter("W1b32", [H, 2, 32], BF16, isOutput=False)
    w2bd_dt = BF16 if SCORE_BF16 else F32
    w2bd_d = nc.declare_dram_parameter("W2BDpad", [128, 248], w2bd_dt, isOutput=False)
    mask_d = nc.declare_dram_parameter("SUmaskB", [128, 132], F32, isOutput=False)
    out_d = nc.declare_dram_parameter("out", [S, H], F32, isOutput=True)

    with tile.TileContext(nc) as tc:
        with (
            tc.tile_pool(name="consts", bufs=1) as consts,
            tc.tile_pool(name="u", bufs=3) as upool,
            tc.tile_pool(name="e", bufs=1) as epool,
            tc.tile_pool(name="o", bufs=3) as opool,
            tc.tile_pool(name="psb", bufs=2, space="PSUM") as ps_big,
            tc.tile_pool(name="pss", bufs=1, space="PSUM") as ps_small,
        ):
            # ---- load inputs: only SP + ACT have HW DGE queues on TRN2;
            # critical loads split across both, bulk loads on gpsimd SWDGE
            xT = consts.tile([H, S], BF16)
            nc.sync.dma_start(out=xT[:, 0:512], in_=xT_d[:, 0:512])
            nc.scalar.dma_start(out=xT[:, 512:S], in_=xT_d[:, 512:S])
            w1ra = consts.tile([H, 128], BF16)
            nc.sync.dma_start(out=w1ra, in_=w1ra_d[:, :])
            w1b32 = consts.tile([H, 2, 32], BF16)
            nc.scalar.dma_start(out=w1b32, in_=w1b_d[:, :, :])
            w2pad = consts.tile([128, 248], w2bd_dt)
            nc.scalar.dma_start(out=w2pad, in_=w2bd_d[:, :])
            # mask + b1 + a zero bias column ride one fast 528B/partition DMA
            maskb1 = consts.tile([128, 132], F32)
            nc.sync.dma_start(out=maskb1, in_=mask_d[:, :])
            b1r = maskb1[:, 128:129]
            zbias = maskb1[:, 129:130]

            # warm the PE clock (HAM un-throttles after ~3.4us of sustained
            # work) and preload the exp/tanh ACT table while DMAs run
            scratch = consts.tile([128, 1], F32)
            nc.vector.memset(scratch, 0.0)
            nc.scalar.activation(out=scratch, in_=scratch, func=FT.Tanh)
            wsrc = consts.tile([128, 512], BF16)
            nc.vector.memset(wsrc, 0.0)
            wps = ps_small.tile([128, 512], F32, tag="po0", name="warm_ps")
            for _ in range(6):
                nc.tensor.matmul(
                    out=wps[:, :],
                    lhsT=wsrc[:, 0:128],
                    rhs=wsrc[:, :],
                    start=True,
                    stop=True,
                )

            xaug = consts.tile([128, 8, XAUG_W], BF16)
            nc.gpsimd.dma_start(
                out=xaug[:, :, :],
                in_=xaug_d[:, :].rearrange("(g p) w -> p g w", p=128),
            )

            # ---- FB[p, i] = sum_h W1repA[h, p] * xT[h, i] ----
            FB = consts.tile([128, S], BF16)
            fbp = ps_big.tile([128, S], F32, tag="big")
            for c in range(2):
                sl = slice(c * 512, (c + 1) * 512)
                nc.tensor.matmul(
                    out=fbp[:, sl],
                    lhsT=w1ra[:, :],
                    rhs=xT[:, sl],
                    start=True,
                    stop=True,
                )
            nc.scalar.copy(out=FB[:, 0:512], in_=fbp[:, 0:512])
            nc.scalar.copy(out=FB[:, 512:S], in_=fbp[:, 512:S])

            # ---- G[j8*16+a, jb] = sum_h W1b[h, a] * xT[h, 8*jb+j8] + b1[a] ----
            G = consts.tile([128, NBLK], F32)
            gp = ps_big.tile([128, NBLK], F32, tag="big")
            xTg = xT[:, :].rearrange("h (j e) -> h j e", e=8)
            for q in range(4):
                for r in range(2):
                    nc.tensor.matmul(
                        out=gp[32 * q : 32 * (q + 1), :],
                        lhsT=w1b32[:, r, :],
                        rhs=xTg[:, :, 2 * q + r],
                        start=(r == 0),
                        stop=(r == 1),
                        tile_position=(0, 32 * q),
                    )
            nc.vector.tensor_scalar_add(out=G, in0=gp, scalar1=b1r)

            # ---- out-matmul bookkeeping (interleaved into the main loop;
            # 4 rotating PSUM tiles: ib and ib+4 share tag po{ib%4}) ----
            e_tiles = []
            po_tiles = {}
            next_term = {}  # ib -> next supertile index to accumulate
            active = []

            def activate_ib(ib):
                po_tiles[ib] = ps_small.tile(
                    [128, XAUG_W], F32, tag=f"po{ib % 4}", name=f"po_{ib}"
                )
                next_term[ib] = 0
                active.append(ib)

            def finish_ib(ib):
                po = po_tiles[ib]
                rec = opool.tile([128, 1], F32, tag="rec")
                nc.vector.tensor_scalar_add(
                    out=rec, in0=po[:, H : H + 1], scalar1=1e-10
                )
                nc.vector.reciprocal(out=rec, in_=rec)
                osb = opool.tile([128, H], F32, tag="osb")
                nc.vector.tensor_scalar_mul(out=osb, in0=po[:, 0:H], scalar1=rec)
                nc.sync.dma_start(
                    out=out_d[ib * 128 : (ib + 1) * 128, :], in_=osb
                )
                active.remove(ib)
                if ib + 4 < 8:
                    activate_ib(ib + 4)

            def emit_out_terms(g):
                # out[i,:] = sum_j e[j,i]*x_aug[j]; accumulate terms whose
                # e-supertile is ready, for every ib with a live PSUM slot
                for ib in sorted(active):
                    while next_term[ib] <= min(ib, g):
                        g2 = next_term[ib]
                        col0 = 128 * (ib - g2)
                        nc.tensor.matmul(
                            out=po_tiles[ib][:, :],
                            lhsT=e_tiles[g2][:, col0 : col0 + 128],
                            rhs=xaug[:, g2, :],
                            start=(g2 == 0),
                            stop=(g2 == ib),
                        )
                        next_term[ib] += 1
                    if next_term[ib] > ib:
                        finish_ib(ib)

            for ib in range(4):
                activate_ib(ib)

            # ---- main loop: supertiles of 16 j-blocks (128 j's) ----
            # exp for supertile g is emitted after the first tanh group of
            # supertile g+1: ACT's queue is FIFO, and an exp emitted right
            # after g's last tanh would stall ACT waiting on g's last
            # score matmuls
            pending_exp = None

            def emit_exp(ps, g, Lg):
                e = epool.tile([128, Lg], BF16, tag=f"e{g}", name=f"e_{g}")
                nc.scalar.activation(
                    out=e[:, :], in_=ps[:, :], func=FT.Exp, bias=zbias, scale=1.0
                )
                nc.vector.tensor_mul(e[:, 0:128], e[:, 0:128], maskb1[:, 0:128])
                e_tiles.append(e)
                emit_out_terms(g)

            for g in range(8):
                Lg = S - 128 * g  # psum supertile covers columns i in [128g, S)
                ps = ps_big.tile([128, Lg], F32, tag="big")
                # ramp-up: small leading tanh groups so ACT starts early
                group_sizes = [1, 1, 2, 4, 8] if g == 0 else [8, 8]
                done = 0
                for gi, gs in enumerate(group_sizes):
                    jbs = [16 * g + done + k for k in range(gs)]
                    done += gs
                    if gi == 1 and pending_exp is not None:
                        emit_exp(*pending_exp)
                        pending_exp = None
                    offs = []
                    flat = 0
                    for jb in jbs:
                        offs.append(flat)
                        flat += S - 8 * jb
                    U = upool.tile([128, flat], BF16, tag="u")
                    for jb, o in zip(jbs, offs):
                        Lb = S - 8 * jb
                        nc.vector.tensor_scalar_add(
                            out=U[:, o : o + Lb],
                            in0=FB[:, 8 * jb : S],
                            scalar1=G[:, jb : jb + 1],
                        )
                    # tanh output fp16 so score matmuls stream 1 col/cycle
                    TT = upool.tile([128, flat], BF16, tag="tt")
                    nc.scalar.activation(out=TT[:, :], in_=U[:, :], func=FT.Tanh)
                    # score matmuls: M=128 sliding-window block-diag weights
                    # (full-width weights enable fast-weight-load; out base
                    # partition always 0; k=0 zero-inits the whole supertile
                    # because its weight columns outside block 0 are zero)
                    for jb, o in zip(jbs, offs):
                        k = jb - 16 * g  # block index within supertile
                        rel0 = 8 * jb - 128 * g  # == 8k
                        lhs_ap = w2pad[:, 120 - 8 * k : 248 - 8 * k]
                        bounds = (
                            [rel0] + [b for b in (512,) if rel0 < b < Lg] + [Lg]
                        )
                        for c0, c1 in zip(bounds[:-1], bounds[1:]):
                            nc.tensor.matmul(
                                out=ps[:, c0:c1],
                                lhsT=lhs_ap,
                                rhs=TT[:, o + (c0 - rel0) : o + (c1 - rel0)],
                                start=(k == 0),
                                stop=(k == 15),
                            )
                pending_exp = (ps, g, Lg)
            emit_exp(*pending_exp)

    nc.compile()
    return nc


_NC_CACHE = None


def _get_nc():
    global _NC_CACHE
    if _NC_CACHE is None:
        _NC_CACHE = _build_nc()
    return _NC_CACHE


def _host_prep(x, W1, b1, w2, b2):
    """Build the per-core input maps (all small derived tensors + shards)."""
    x = np.asarray(x, dtype=np.float32)
    W1 = np.asarray(W1, dtype=np.float32)
    b1 = np.asarray(b1, dtype=np.float32).reshape(-1)
    w2 = np.asarray(w2, dtype=np.float32).reshape(-1)
    b2 = np.asarray(b2, dtype=np.float32).reshape(-1)

    p = np.arange(128)
    W1repA = np.ascontiguousarray(W1[:H][:, p % A]).astype(np.float16)  # [H, 128]
    # W1b32[h, r, m] places g-matmul outputs for j8 = 2q+r at rows 16r+a
    W1b32 = np.zeros((H, 2, 32), dtype=np.float16)
    for r in range(2):
        W1b32[:, r, 16 * r : 16 * r + A] = W1[H:]
    # sliding-window block-diag weights: W2BDpad[p, 120 + j8] = w2[a]
    # (lhsT for block k is W2BDpad[:, 120-8k : 248-8k])
    W2BDpad = np.zeros((128, 248), dtype=np.float32)
    W2BDpad[p, 120 + p // A] = w2[p % A]
    if SCORE_BF16:
        W2BDpad = W2BDpad.astype(np.float16)
    # strictly-upper mask plus b1 (col 128) and a zero exp-bias col (129)
    SUmaskB = np.zeros((128, 132), dtype=np.float32)
    SUmaskB[:, 0:128] = p[:, None] < p[None, :]
    SUmaskB[:, 128] = b1[p % A]

    shared = {
        "W1repA": W1repA,
        "W1b32": W1b32,
        "W2BDpad": W2BDpad,
        "SUmaskB": SUmaskB,
    }
    in_maps = []
    for c in range(NCORES):
        xb = x[c]  # [S, H]
        x_aug = np.zeros((S, XAUG_W), dtype=np.float16)
        x_aug[:, :H] = xb
        x_aug[:, H] = 1.0
        m = dict(shared)
        m["x_aug"] = x_aug
        m["xT"] = np.ascontiguousarray(xb.T).astype(np.float16)
        in_maps.append(m)
    return in_maps


def kernel(x, W1, b1, w2, b2, _trace=False):
    nc = _get_nc()
    in_maps = _host_prep(x, W1, b1, w2, b2)
    res = run_bass_kernel_spmd(nc, in_maps, list(range(NCORES)), trace=_trace)
    out = np.stack([np.asarray(res.results[c]["out"]) for c in range(NCORES)])
    if _trace:
        kernel.last_exec_time_ns = res.exec_time_ns
        kernel.last_profile = res.profile_json
    return out



# revision 2
# speedup vs baseline: 2.6923x; 2.6923x over previous
"""Concatenation (additive/Bahdanau-style) attention Trainium2 kernel.

Math (per batch b):
    f = x @ W1[:H]          # [S, A]
    g = x @ W1[H:] + b1     # [S, A]
    scores[i, j] = w2 . tanh(f[i] + g[j]) + b2
    e = exp(scores) * (j < i)
    out[i] = sum_j e[i, j] x[j] / (sum_j e[i, j] + 1e-10)

Sharding: data-parallel over batch, one batch element per NeuronCore (B=8).

Key idea: the pairwise tanh is separable to high accuracy.  With
T=tanh(f), G=tanh(g) we have tanh(f+g) = (T+G)/(1+TG), a smooth 2-D
function whose Gaussian-weighted SVD decays exponentially.  We use a
skeleton (cross) approximation with basis functions tanh(.+node_k):

    tanh(f+g) ~= sum_{k,l} tanh(f+node_k) * M[k,l] * tanh(g+node_l)

(k=0 is a constant feature, tanh(arg+20)=1).  The fit matrix M is
input-independent (fit offline on a Gaussian-weighted grid; fitting is
done modulo additive functions of f, which cancel in the row softmax).
Then

    scores[i,j] ~= sum_{a,k} U[(a,k), i] * Vt[(a,k), j]

with U[(a,k), i] = tanh(f_ia + node_k) and
Vt[(a,k), j] = w2_a * sum_l M[k,l] tanh(g_ja + b1_a + node_l):
one PE matmul per 128-row j-supertile replaces the S^2*A/2 pairwise
tanh evaluations (60us of ACT time) entirely.

Per-core schedule:
  - replicate matmuls: lhsT columns (a,k) = W1f[:,a] (resp W1g) repeated
    per node slot -> PSUM [128/112, S]; one ACT tanh per 512-col piece
    with per-partition bias = node_k (U side; 20.0 for the const row)
    or node_l + b1_a (V side) -> fp16 features in SBUF.
  - fold matmul: block-diag FOLD[(a,l),(a,k)] = w2_a*M[k,l] -> PSUM,
    DVE-copied to fp16 Vt (lhsT of the score matmuls).
  - supertile g (j in [128g,128g+128), i in [128g, S)): score matmul
    contracting all 128 (a,k) rows -> PSUM [128, Lg]; ACT exp
    (bias=0; b2 cancels in softmax); strictly-upper fp16 mask on the
    diagonal 128-chunk (DVE) enforces j < i.
  - out: for each 128-row i-block ib, accumulate matmuls over
    supertiles g<=ib: lhsT = e_g[:, i-cols] (K=j), rhs = x_aug (x with
    a ones column) so the softmax denominator falls out of the same
    matmuls; then reciprocal+scale (DVE) and DMA out.
  - exp(g) is emitted after scores(g+1) so PE streams scores g+1 while
    ACT exponentiates supertile g.
"""

import numpy as np

import concourse.bass as bass
import concourse.tile as tile
from concourse import bacc, mybir
from concourse.bass_utils import run_bass_kernel_spmd

B, S, H, A = 8, 1024, 128, 16
NCORES = 8
XAUG_W = H + 4  # x plus a ones column, padded to 132 floats

FT = mybir.ActivationFunctionType
F32 = mybir.dt.float32
F16 = mybir.dt.float16  # fp16: same 1 col/cycle as bf16, 8x the mantissa

K = 7            # tanh nodes per hidden channel
KP1 = K + 1      # + one constant feature per channel
DU = A * KP1     # 128: score-matmul contraction dim (exactly fills PE)
DV = A * K       # 112 raw V rows
NODES = np.array([-2.3907, -1.2389, -0.4027, 0.0, 0.4027, 1.2389, 2.3907])


def _fit_M():
    """Weighted LSQ fit of tanh(f+g) in the tanh(.+node) skeleton basis.

    Fit is modulo additive functions of f (V-side basis and target are
    centered along g): those cancel in the softmax over j.  Input-
    independent; computed once at import (pure numpy, ~10ms).
    """
    n = 1101
    xg = np.linspace(-5.0, 5.0, n)
    Yg = np.tanh(xg[:, None] + xg[None, :])
    w = np.exp(-(xg ** 2) / 2.0)  # sigma=1 (actual f,g sigma ~0.71)
    w = w + 2e-3 * w.max()        # floor so corners stay sane
    w /= w.sum()
    sw = np.sqrt(w)
    Au = np.concatenate(
        [np.ones((n, 1)), np.tanh(xg[:, None] + NODES[None, :])], axis=1
    )
    Bv = np.tanh(NODES[None, :] + xg[:, None])
    Bc = Bv - (Bv * w[:, None]).sum(0, keepdims=True)
    Yc = Yg - (Yg * w[None, :]).sum(1, keepdims=True)

    def pinvr(Aw, r=1e-7):
        U_, S_, Vt_ = np.linalg.svd(Aw, full_matrices=False)
        return (Vt_.T * (S_ / (S_ ** 2 + r * S_[0] ** 2))) @ U_.T

    return pinvr(Au * sw[:, None]) @ (Yc * sw[:, None] * sw[None, :]) @ pinvr(
        Bc * sw[:, None]
    ).T  # [KP1, K]


M_FIT = _fit_M()


def _build_nc():
    nc = bacc.Bacc(None)

    xT_d = nc.declare_dram_parameter("xT", [H, S], F16, isOutput=False)
    xaug_d = nc.declare_dram_parameter("x_aug", [S, XAUG_W], F16, isOutput=False)
    ww_d = nc.declare_dram_parameter("WW", [H, DU + DV], F16, isOutput=False)
    fold_d = nc.declare_dram_parameter("FOLD", [DV, DU], F16, isOutput=False)
    bias_d = nc.declare_dram_parameter("BIASM", [128, 4], F32, isOutput=False)
    mask_d = nc.declare_dram_parameter("MASKF", [128, 128], F16, isOutput=False)
    out_d = nc.declare_dram_parameter("out", [S, H], F32, isOutput=True)

    with tile.TileContext(nc) as tc:
        with (
            tc.tile_pool(name="consts", bufs=1) as consts,
            tc.tile_pool(name="e", bufs=1) as epool,
            tc.tile_pool(name="o", bufs=3) as opool,
            tc.tile_pool(name="psb", bufs=2, space="PSUM") as ps_big,
            tc.tile_pool(name="pss", bufs=1, space="PSUM") as ps_small,
        ):
            # ---- input loads: SP + ACT HW DGE queues for critical tensors,
            # gpsimd SWDGE for the bulk x_aug (needed late, by out matmuls)
            xT = consts.tile([H, S], F16)
            nc.sync.dma_start(out=xT[:, 0:512], in_=xT_d[:, 0:512])
            nc.scalar.dma_start(out=xT[:, 512:S], in_=xT_d[:, 512:S])
            ww = consts.tile([H, DU + DV], F16)
            nc.scalar.dma_start(out=ww, in_=ww_d[:, :])
            fold = consts.tile([DV, DU], F16)
            nc.scalar.dma_start(out=fold, in_=fold_d[:, :])
            biasm = consts.tile([128, 4], F32)
            nc.sync.dma_start(out=biasm, in_=bias_d[:, :])
            maskf = consts.tile([128, 128], F16)
            nc.sync.dma_start(out=maskf, in_=mask_d[:, :])

            ubias = biasm[0:DU, 0:1]
            vbias = biasm[0:DV, 1:2]
            zbias = biasm[:, 2:3]

            # warm the PE clock (HAM un-throttles after sustained work) and
            # preload the exp/tanh ACT table while the input DMAs run
            scratch = consts.tile([128, 1], F32)
            nc.vector.memset(scratch, 0.0)
            nc.scalar.activation(out=scratch, in_=scratch, func=FT.Tanh)
            wsrc = consts.tile([128, 512], F16)
            nc.vector.memset(wsrc, 0.0)
            wps = ps_small.tile([128, 512], F32, tag="po0", name="warm_ps")
            for _ in range(6):
                nc.tensor.matmul(
                    out=wps[:, :],
                    lhsT=wsrc[:, 0:128],
                    rhs=wsrc[:, :],
                    start=True,
                    stop=True,
                )

            xaug = consts.tile([128, 8, XAUG_W], F16)
            nc.gpsimd.dma_start(
                out=xaug[:, :, :],
                in_=xaug_d[:, :].rearrange("(g p) w -> p g w", p=128),
            )

            # ---- V features first (they feed the longer fold+copy chain)
            # raw: Vraw[(a,l), j] = tanh(g_ja + b1_a + node_l)
            psV = ps_big.tile([DV, S], F32, tag="big")
            for c in range(2):
                sl = slice(512 * c, 512 * (c + 1))
                nc.tensor.matmul(
                    out=psV[:, sl],
                    lhsT=ww[:, DU : DU + DV],
                    rhs=xT[:, sl],
                    start=True,
                    stop=True,
                )
            Vraw = consts.tile([DV, S], F16)
            for c in range(2):
                sl = slice(512 * c, 512 * (c + 1))
                nc.scalar.activation(
                    out=Vraw[:, sl], in_=psV[:, sl], func=FT.Tanh,
                    bias=vbias, scale=1.0,
                )

            # ---- U features: U[(a,k), i] = tanh(f_ia + node_k); k=0 const
            psU = ps_big.tile([DU, S], F32, tag="big")
            for c in range(2):
                sl = slice(512 * c, 512 * (c + 1))
                nc.tensor.matmul(
                    out=psU[:, sl],
                    lhsT=ww[:, 0:DU],
                    rhs=xT[:, sl],
                    start=True,
                    stop=True,
                )
            U = consts.tile([DU, S], F16)
            for c in range(2):
                sl = slice(512 * c, 512 * (c + 1))
                nc.scalar.activation(
                    out=U[:, sl], in_=psU[:, sl], func=FT.Tanh,
                    bias=ubias, scale=1.0,
                )

            # ---- fold: Vt[(a,k), j] = w2_a sum_l M[k,l] Vraw[(a,l), j]
            # (512-col PSUM pieces on the po1/po2 banks; the main loop's
            # first use of those banks comes well after the copies drain)
            Vt = consts.tile([DU, S], F16)
            for c in range(2):
                sl = slice(512 * c, 512 * (c + 1))
                psF = ps_small.tile([DU, 512], F32, tag=f"po{c + 1}",
                                    name=f"psF{c}")
                nc.tensor.matmul(
                    out=psF[:, :], lhsT=fold[:, :], rhs=Vraw[:, sl],
                    start=True, stop=True,
                )
                if c == 0:
                    # the g=0 score matmul only needs columns 0:128
                    nc.vector.tensor_copy(Vt[:, 0:128], psF[:, 0:128])
                    nc.vector.tensor_copy(Vt[:, 128:512], psF[:, 128:512])
                else:
                    nc.vector.tensor_copy(Vt[:, sl], psF[:, :])

            # ---- out-matmul bookkeeping (interleaved into the main loop;
            # 4 rotating PSUM tiles: ib and ib+4 share tag po{ib%4})
            e_tiles = []
            po_tiles = {}
            next_term = {}  # ib -> next supertile index to accumulate
            active = []

            def activate_ib(ib):
                po_tiles[ib] = ps_small.tile(
                    [128, XAUG_W], F32, tag=f"po{ib % 4}", name=f"po_{ib}"
                )
                next_term[ib] = 0
                active.append(ib)

            def finish_ib(ib):
                po = po_tiles[ib]
                rec = opool.tile([128, 1], F32, tag="rec")
                nc.vector.tensor_scalar_add(
                    out=rec, in0=po[:, H : H + 1], scalar1=1e-10
                )
                nc.vector.reciprocal(out=rec, in_=rec)
                osb = opool.tile([128, H], F32, tag="osb")
                nc.vector.tensor_scalar_mul(out=osb, in0=po[:, 0:H], scalar1=rec)
                nc.sync.dma_start(
                    out=out_d[ib * 128 : (ib + 1) * 128, :], in_=osb
                )
                active.remove(ib)
                if ib + 4 < 8:
                    activate_ib(ib + 4)

            def emit_out_terms(g):
                # out[i,:] = sum_j e[j,i]*x_aug[j]; accumulate terms whose
                # e-supertile is ready, for every ib with a live PSUM slot
                for ib in sorted(active):
                    while next_term[ib] <= min(ib, g):
                        g2 = next_term[ib]
                        col0 = 128 * (ib - g2)
                        nc.tensor.matmul(
                            out=po_tiles[ib][:, :],
                            lhsT=e_tiles[g2][:, col0 : col0 + 128],
                            rhs=xaug[:, g2, :],
                            start=(g2 == 0),
                            stop=(g2 == ib),
                        )
                        next_term[ib] += 1
                    if next_term[ib] > ib:
                        finish_ib(ib)

            for ib in range(4):
                activate_ib(ib)

            # ---- main loop over supertiles (128 j's each); exp(g) emitted
            # after scores(g+1) so ACT and PE pipeline
            pending = None

            def emit_exp(ps, g, Lg):
                e = epool.tile([128, Lg], F16, tag=f"e{g}", name=f"e_{g}")
                nc.scalar.activation(
                    out=e[:, :], in_=ps[:, :], func=FT.Exp, bias=zbias,
                    scale=1.0,
                )
                nc.vector.tensor_mul(e[:, 0:128], e[:, 0:128], maskf)
                e_tiles.append(e)
                emit_out_terms(g)

            for g in range(8):
                Lg = S - 128 * g
                ps = ps_big.tile([128, Lg], F32, tag="big")
                bounds = [0] + ([512] if Lg > 512 else []) + [Lg]
                for c0, c1 in zip(bounds[:-1], bounds[1:]):
                    nc.tensor.matmul(
                        out=ps[:, c0:c1],
                        lhsT=Vt[:, 128 * g : 128 * (g + 1)],
                        rhs=U[:, 128 * g + c0 : 128 * g + c1],
                        start=True,
                        stop=True,
                    )
                if pending is not None:
                    emit_exp(*pending)
                pending = (ps, g, Lg)
            emit_exp(*pending)

    nc.compile()
    return nc


_NC_CACHE = None


def _get_nc():
    global _NC_CACHE
    if _NC_CACHE is None:
        _NC_CACHE = _build_nc()
    return _NC_CACHE


def _host_prep(x, W1, b1, w2, b2):
    """Build the per-core input maps (small derived tensors + shards)."""
    x = np.asarray(x, dtype=np.float32)
    W1 = np.asarray(W1, dtype=np.float32)
    b1 = np.asarray(b1, dtype=np.float32).reshape(-1)
    w2 = np.asarray(w2, dtype=np.float32).reshape(-1)

    W1f, W1g = W1[:H], W1[H:]  # [H, A] each
    WW = np.zeros((H, DU + DV), dtype=np.float16)
    FOLD = np.zeros((DV, DU), dtype=np.float16)
    BIASM = np.zeros((128, 4), dtype=np.float32)
    for a in range(A):
        WW[:, a * KP1 : (a + 1) * KP1] = W1f[:, a : a + 1]
        WW[:, DU + a * K : DU + (a + 1) * K] = W1g[:, a : a + 1]
        FOLD[a * K : (a + 1) * K, a * KP1 : (a + 1) * KP1] = w2[a] * M_FIT.T
        BIASM[a * KP1, 0] = 20.0  # const feature: tanh(f+20) == 1
        BIASM[a * KP1 + 1 : (a + 1) * KP1, 0] = NODES
        BIASM[a * K : (a + 1) * K, 1] = NODES + b1[a]
    p = np.arange(128)
    MASKF = (p[:, None] < p[None, :]).astype(np.float16)

    shared = {"WW": WW, "FOLD": FOLD, "BIASM": BIASM, "MASKF": MASKF}
    in_maps = []
    for c in range(NCORES):
        xb = x[c]  # [S, H]
        x_aug = np.zeros((S, XAUG_W), dtype=np.float16)
        x_aug[:, :H] = xb
        x_aug[:, H] = 1.0
        m = dict(shared)
        m["x_aug"] = x_aug
        m["xT"] = np.ascontiguousarray(xb.T).astype(np.float16)
        in_maps.append(m)
    return in_maps


def kernel(x, W1, b1, w2, b2, _trace=False):
    nc = _get_nc()
    in_maps = _host_prep(x, W1, b1, w2, b2)
    res = run_bass_kernel_spmd(nc, in_maps, list(range(NCORES)), trace=_trace)
    out = np.stack([np.asarray(res.results[c]["out"]) for c in range(NCORES)])
    if _trace:
        kernel.last_exec_time_ns = res.exec_time_ns
        kernel.last_profile = res.profile_json
    return out


# revision 6
# speedup vs baseline: 2.9386x; 1.0915x over previous
"""Concatenation (additive/Bahdanau-style) attention Trainium2 kernel.

Math (per batch b):
    f = x @ W1[:H]          # [S, A]
    g = x @ W1[H:] + b1     # [S, A]
    scores[i, j] = w2 . tanh(f[i] + g[j]) + b2
    e = exp(scores) * (j < i)
    out[i] = sum_j e[i, j] x[j] / (sum_j e[i, j] + 1e-10)

Sharding: data-parallel over batch, one batch element per NeuronCore (B=8).

Key idea: the pairwise tanh is separable to high accuracy.  With
T=tanh(f), G=tanh(g) we have tanh(f+g) = (T+G)/(1+TG), a smooth 2-D
function whose Gaussian-weighted SVD decays exponentially.  We use a
skeleton (cross) approximation with basis functions tanh(.+node_k):

    tanh(f+g) ~= sum_{k,l} tanh(f+node_k) * M[k,l] * tanh(g+node_l)

(k=0 is a constant feature, tanh(arg+20)=1).  The fit matrix M is
input-independent (fit offline on a Gaussian-weighted grid; fitting is
done modulo additive functions of f, which cancel in the row softmax).
Then

    scores[i,j] ~= sum_{a,k} U[(a,k), i] * Vt[(a,k), j]

with U[(a,k), i] = tanh(f_ia + node_k) and
Vt[(a,k), j] = w2_a * sum_l M[k,l] tanh(g_ja + b1_a + node_l):
one PE matmul per 128-row j-supertile replaces the S^2*A/2 pairwise
tanh evaluations (60us of ACT time) entirely.

Per-core schedule:
  - replicate matmuls: lhsT columns (a,k) = W1f[:,a] (resp W1g) repeated
    per node slot -> PSUM [128/112, S]; one ACT tanh per 512-col piece
    with per-partition bias = node_k (U side; 20.0 for the const row)
    or node_l + b1_a (V side) -> fp16 features in SBUF.
  - fold matmul: block-diag FOLD[(a,l),(a,k)] = w2_a*M[k,l] -> PSUM,
    DVE-copied to fp16 Vt (lhsT of the score matmuls).
  - supertile g (j in [128g,128g+128), i in [128g, S)): score matmul
    contracting all 128 (a,k) rows -> PSUM [128, Lg]; ACT exp
    (bias=0; b2 cancels in softmax); strictly-upper fp16 mask on the
    diagonal 128-chunk (DVE) enforces j < i.
  - out: for each 128-row i-block ib, accumulate matmuls over
    supertiles g<=ib: lhsT = e_g[:, i-cols] (K=j), rhs = x_aug (x with
    a ones column) so the softmax denominator falls out of the same
    matmuls; then reciprocal+scale (DVE) and DMA out.
  - exp(g) is emitted after scores(g+1) so PE streams scores g+1 while
    ACT exponentiates supertile g.
"""

import numpy as np

import concourse.bass as bass
import concourse.tile as tile
from concourse import bacc, mybir
from concourse.bass_utils import run_bass_kernel_spmd

B, S, H, A = 8, 1024, 128, 16
NCORES = 8
XAUG_W = H + 4  # x plus a ones column, padded to 132 floats

FT = mybir.ActivationFunctionType
F32 = mybir.dt.float32
F16 = mybir.dt.float16  # fp16: same 1 col/cycle as bf16, 8x the mantissa

K = 7            # tanh nodes per hidden channel
KP1 = K + 1      # + one constant feature per channel
DU = A * KP1     # 128: score-matmul contraction dim (exactly fills PE)
DV = A * K       # 112 raw V rows
NODES = np.array([-2.3907, -1.2389, -0.4027, 0.0, 0.4027, 1.2389, 2.3907])


def _fit_M():
    """Weighted LSQ fit of tanh(f+g) in the tanh(.+node) skeleton basis.

    Fit is modulo additive functions of f (V-side basis and target are
    centered along g): those cancel in the softmax over j.  Input-
    independent; computed once at import (pure numpy, ~10ms).
    """
    n = 1101
    xg = np.linspace(-5.0, 5.0, n)
    Yg = np.tanh(xg[:, None] + xg[None, :])
    w = np.exp(-(xg ** 2) / 2.0)  # sigma=1 (actual f,g sigma ~0.71)
    w = w + 2e-3 * w.max()        # floor so corners stay sane
    w /= w.sum()
    sw = np.sqrt(w)
    Au = np.concatenate(
        [np.ones((n, 1)), np.tanh(xg[:, None] + NODES[None, :])], axis=1
    )
    Bv = np.tanh(NODES[None, :] + xg[:, None])
    Bc = Bv - (Bv * w[:, None]).sum(0, keepdims=True)
    Yc = Yg - (Yg * w[None, :]).sum(1, keepdims=True)

    def pinvr(Aw, r=1e-7):
        U_, S_, Vt_ = np.linalg.svd(Aw, full_matrices=False)
        return (Vt_.T * (S_ / (S_ ** 2 + r * S_[0] ** 2))) @ U_.T

    return pinvr(Au * sw[:, None]) @ (Yc * sw[:, None] * sw[None, :]) @ pinvr(
        Bc * sw[:, None]
    ).T  # [KP1, K]


M_FIT = _fit_M()


def _build_nc():
    nc = bacc.Bacc(None)

    # consts pack (f16): cols [0:DU+DV)=WW replicate weights,
    # [DU+DV : DU+DV+DU) = FOLD (rows 0:DV), last 128 = strictly-upper mask
    CP_W = DU + DV + DU + 128
    xT_d = nc.declare_dram_parameter("xT", [H, S], F16, isOutput=False)
    xaug_d = nc.declare_dram_parameter("x_aug", [S, XAUG_W], F16, isOutput=False)
    cp_d = nc.declare_dram_parameter("CPACK", [128, CP_W], F16, isOutput=False)
    bias_d = nc.declare_dram_parameter("BIASM", [128, 4], F32, isOutput=False)
    out_d = nc.declare_dram_parameter("out", [S, H], F32, isOutput=True)

    with tile.TileContext(nc) as tc:
        with (
            tc.tile_pool(name="consts", bufs=1) as consts,
            tc.tile_pool(name="e", bufs=1) as epool,
            tc.tile_pool(name="o", bufs=3) as opool,
            tc.tile_pool(name="psb", bufs=2, space="PSUM") as ps_big,
            tc.tile_pool(name="pss", bufs=1, space="PSUM") as ps_small,
        ):
            # ---- input loads: each DMA op blocks its engine's queue for the
            # transfer, so order by when the data is needed.  Both HW DGE
            # queues (SP, ACT); no gpsimd SWDGE (its multi-us drain blocks
            # dependents).  PE can start once CPACK + xT[:,0:512] land.
            cpack = consts.tile([128, CP_W], F16)
            nc.sync.dma_start(out=cpack, in_=cp_d[:, :])
            xT = consts.tile([H, S], F16)
            nc.scalar.dma_start(out=xT[:, 0:512], in_=xT_d[:, 0:512])
            nc.sync.dma_start(out=xT[:, 512:S], in_=xT_d[:, 512:S])
            biasm = consts.tile([128, 4], F32)
            nc.scalar.dma_start(out=biasm, in_=bias_d[:, :])
            xaug = consts.tile([128, 8, XAUG_W], F16)
            nc.sync.dma_start(
                out=xaug[:, :, :],
                in_=xaug_d[:, :].rearrange("(g p) w -> p g w", p=128),
            )

            ww = cpack[:, 0 : DU + DV]
            fold = cpack[0:DV, DU + DV : DU + DV + DU]
            maskf = cpack[:, DU + DV + DU : CP_W]
            ubias = biasm[0:DU, 0:1]
            vbias = biasm[0:DV, 1:2]
            zbias = biasm[:, 2:3]

            # preload the exp/tanh ACT table while the input DMAs run (the
            # auto-inserted ACT_TABLE_LOAD is async; issuing a dummy tanh
            # first makes it overlap the DMAs instead of the first real tanh)
            scratch = consts.tile([128, 1], F32)
            nc.vector.memset(scratch, 0.0)
            nc.scalar.activation(out=scratch, in_=scratch, func=FT.Tanh)

            # ---- V features first (they feed the longer fold+copy chain)
            # raw: Vraw[(a,l), j] = tanh(g_ja + b1_a + node_l)
            psV = ps_big.tile([DV, S], F32, tag="big")
            for c in range(2):
                sl = slice(512 * c, 512 * (c + 1))
                nc.tensor.matmul(
                    out=psV[:, sl],
                    lhsT=ww[:, DU : DU + DV],
                    rhs=xT[:, sl],
                    start=True,
                    stop=True,
                )
            Vraw = consts.tile([DV, S], F16)
            for c in range(2):
                sl = slice(512 * c, 512 * (c + 1))
                nc.scalar.activation(
                    out=Vraw[:, sl], in_=psV[:, sl], func=FT.Tanh,
                    bias=vbias, scale=1.0,
                )

            # ---- U features: U[(a,k), i] = tanh(f_ia + node_k); k=0 const
            psU = ps_big.tile([DU, S], F32, tag="big")
            for c in range(2):
                sl = slice(512 * c, 512 * (c + 1))
                nc.tensor.matmul(
                    out=psU[:, sl],
                    lhsT=ww[:, 0:DU],
                    rhs=xT[:, sl],
                    start=True,
                    stop=True,
                )
            U = consts.tile([DU, S], F16)
            for c in range(2):
                sl = slice(512 * c, 512 * (c + 1))
                nc.scalar.activation(
                    out=U[:, sl], in_=psU[:, sl], func=FT.Tanh,
                    bias=ubias, scale=1.0,
                )

            # ---- fold: Vt[(a,k), j] = w2_a sum_l M[k,l] Vraw[(a,l), j]
            # (512-col PSUM pieces on the po1/po2 banks; the main loop's
            # first use of those banks comes well after the copies drain)
            Vt = consts.tile([DU, S], F16)
            for c in range(2):
                sl = slice(512 * c, 512 * (c + 1))
                psF = ps_small.tile([DU, 512], F32, tag=f"po{c + 1}",
                                    name=f"psF{c}")
                nc.tensor.matmul(
                    out=psF[:, :], lhsT=fold[:, :], rhs=Vraw[:, sl],
                    start=True, stop=True,
                )
                if c == 0:
                    # the g=0 score matmul only needs columns 0:128
                    nc.vector.tensor_copy(Vt[:, 0:128], psF[:, 0:128])
                    nc.vector.tensor_copy(Vt[:, 128:512], psF[:, 128:512])
                else:
                    nc.vector.tensor_copy(Vt[:, sl], psF[:, :])

            # ---- out-matmul bookkeeping (interleaved into the main loop;
            # 4 rotating PSUM tiles: ib and ib+4 share tag po{ib%4})
            e_tiles = []
            po_tiles = {}
            next_term = {}  # ib -> next supertile index to accumulate
            active = []

            def activate_ib(ib):
                po_tiles[ib] = ps_small.tile(
                    [128, XAUG_W], F32, tag=f"po{ib % 4}", name=f"po_{ib}"
                )
                next_term[ib] = 0
                active.append(ib)

            def finish_ib(ib):
                po = po_tiles[ib]
                rec = opool.tile([128, 1], F32, tag="rec")
                if ib == 0:
                    # only row i=0 has an empty sum (reference adds 1e-10)
                    nc.vector.tensor_scalar_add(
                        out=rec, in0=po[:, H : H + 1], scalar1=1e-10
                    )
                    nc.vector.reciprocal(out=rec, in_=rec)
                else:
                    nc.vector.reciprocal(out=rec, in_=po[:, H : H + 1])
                osb = opool.tile([128, H], F32, tag="osb")
                nc.vector.tensor_scalar_mul(out=osb, in0=po[:, 0:H], scalar1=rec)
                nc.sync.dma_start(
                    out=out_d[ib * 128 : (ib + 1) * 128, :], in_=osb
                )
                active.remove(ib)
                if ib + 4 < 8:
                    activate_ib(ib + 4)

            def emit_out_terms(g):
                # out[i,:] = sum_j e[j,i]*x_aug[j]; accumulate terms whose
                # e-supertile is ready, for every ib with a live PSUM slot
                for ib in sorted(active):
                    while next_term[ib] <= min(ib, g):
                        g2 = next_term[ib]
                        col0 = 128 * (ib - g2)
                        nc.tensor.matmul(
                            out=po_tiles[ib][:, :],
                            lhsT=e_tiles[g2][:, col0 : col0 + 128],
                            rhs=xaug[:, g2, :],
                            start=(g2 == 0),
                            stop=(g2 == ib),
                        )
                        next_term[ib] += 1
                    if next_term[ib] > ib:
                        finish_ib(ib)

            for ib in range(4):
                activate_ib(ib)

            # ---- main loop over supertiles (128 j's each); exp(g) emitted
            # after scores(g+1) so ACT and PE pipeline
            pending = None

            def emit_exp(ps, g, Lg):
                e = epool.tile([128, Lg], F16, tag=f"e{g}", name=f"e_{g}")
                nc.scalar.activation(
                    out=e[:, :], in_=ps[:, :], func=FT.Exp, bias=zbias,
                    scale=1.0,
                )
                nc.vector.tensor_mul(e[:, 0:128], e[:, 0:128], maskf)
                e_tiles.append(e)
                emit_out_terms(g)

            for g in range(8):
                Lg = S - 128 * g
                ps = ps_big.tile([128, Lg], F32, tag="big")
                bounds = [0] + ([512] if Lg > 512 else []) + [Lg]
                for c0, c1 in zip(bounds[:-1], bounds[1:]):
                    nc.tensor.matmul(
                        out=ps[:, c0:c1],
                        lhsT=Vt[:, 128 * g : 128 * (g + 1)],
                        rhs=U[:, 128 * g + c0 : 128 * g + c1],
                        start=True,
                        stop=True,
                    )
                if pending is not None:
                    emit_exp(*pending)
                pending = (ps, g, Lg)
            emit_exp(*pending)

    nc.compile()
    return nc


_NC_CACHE = None


def _get_nc():
    global _NC_CACHE
    if _NC_CACHE is None:
        _NC_CACHE = _build_nc()
    return _NC_CACHE


def _host_prep(x, W1, b1, w2, b2):
    """Build the per-core input maps (small derived tensors + shards)."""
    x = np.asarray(x, dtype=np.float32)
    W1 = np.asarray(W1, dtype=np.float32)
    b1 = np.asarray(b1, dtype=np.float32).reshape(-1)
    w2 = np.asarray(w2, dtype=np.float32).reshape(-1)

    W1f, W1g = W1[:H], W1[H:]  # [H, A] each
    CP_W = DU + DV + DU + 128
    CPACK = np.zeros((128, CP_W), dtype=np.float16)
    BIASM = np.zeros((128, 4), dtype=np.float32)
    for a in range(A):
        CPACK[:, a * KP1 : (a + 1) * KP1] = W1f[:, a : a + 1]
        CPACK[:, DU + a * K : DU + (a + 1) * K] = W1g[:, a : a + 1]
        CPACK[a * K : (a + 1) * K, DU + DV + a * KP1 : DU + DV + (a + 1) * KP1] = (
            w2[a] * M_FIT.T
        )
        BIASM[a * KP1, 0] = 20.0  # const feature: tanh(f+20) == 1
        BIASM[a * KP1 + 1 : (a + 1) * KP1, 0] = NODES
        BIASM[a * K : (a + 1) * K, 1] = NODES + b1[a]
    p = np.arange(128)
    CPACK[:, DU + DV + DU :] = (p[:, None] < p[None, :]).astype(np.float16)

    shared = {"CPACK": CPACK, "BIASM": BIASM}
    in_maps = []
    for c in range(NCORES):
        xb = x[c]  # [S, H]
        x_aug = np.zeros((S, XAUG_W), dtype=np.float16)
        x_aug[:, :H] = xb
        x_aug[:, H] = 1.0
        m = dict(shared)
        m["x_aug"] = x_aug
        m["xT"] = np.ascontiguousarray(xb.T).astype(np.float16)
        in_maps.append(m)
    return in_maps


def kernel(x, W1, b1, w2, b2, _trace=False):
    nc = _get_nc()
    in_maps = _host_prep(x, W1, b1, w2, b2)
    res = run_bass_kernel_spmd(nc, in_maps, list(range(NCORES)), trace=_trace)
    out = np.stack([np.asarray(res.results[c]["out"]) for c in range(NCORES)])
    if _trace:
        kernel.last_exec_time_ns = res.exec_time_ns
        kernel.last_profile = res.profile_json
    return out
